# revision 8
# baseline (speedup 1.0000x reference)
"""Causal self-attention (B=2, T=2048, C=1024, H=16) on 8 Trainium2 cores.

Sharding: DP2 over batch x TP4 over heads (4 heads/core). Each core computes
its batch's QKV projection for its heads, RoPE, causal attention, and a
partial c_proj over its 256 input channels. Host sums the 4 partials per
batch and adds b_proj.

v3 (from v2 @ 181.7us):
- PE warmup burst (20 dummy matmuls) during the startup DMA wait so the HAM
  clock-gate is at K=8/8 (2.4 GHz) when real matmuls start, instead of
  paying 1.2 GHz for the first ~24us.
- Startup DMA critical path: x0 split across the scalar+vector queues, wqk
  in two m-pair halves on sync; wv moved to vector; xch1 moved to the gated
  gpsimd stream so it can't steal HBM bandwidth from the startup loads.
- Engine rebalance: Scalar keeps only exp (it was 111us busy, near
  co-bottleneck with PE): qk bias-add -> vector tensor_scalar_add, rope
  mul/add -> gpsimd, softmax-normalize muls -> gpsimd, all psum->sbuf
  copies (ysb / ost) -> vector.
- Softmax normalize batched per (chunk, head-pair): one [65,2*SC] psum, one
  ysb copy, one reciprocal, one partition_broadcast (was 2x of each).
- Last chunk's p@v is split into two 256-wide column halves so cproj +
  normalize of half 0 overlap the tail p@v matmuls (kills a 5.5us PE gap).
- Output tiles DMA per 512-half as soon as each copy lands.

Scores are computed two heads at a time via tile_position row packing, exp
runs on ScalarE straight from PSUM with the 1/sqrt(D) scale fused, and the
causal mask is an affine_select on diagonal blocks only. V carries a ones
column per head so the softmax denominator falls out of the p@v matmul.
"""

import sys

sys.path.insert(0, "/opt/trn_rl_repo")

import math

import ml_dtypes
import numpy as np

import concourse.bass as bass
import concourse.mybir as mybir
import concourse.tile as tile
from concourse import bacc, bass_utils

B, T, C = 2, 2048, 1024
H, D = 16, 64
N_CORES = 8
DP, TP = 2, 4
HPC = H // TP  # heads per core
SC = 512  # t-chunk width / psum bank width
NT = T // SC
NSB = T // 128  # s-blocks

F32 = mybir.dt.float32
BF16 = mybir.dt.bfloat16

_cached = {}


def _build_program():
    nc = bacc.Bacc("TRN2", target_bir_lowering=False, debug=False, num_devices=N_CORES)

    xT_d = nc.dram_tensor("xT", [C, T], BF16, kind="ExternalInput").ap()
    wqk_d = nc.dram_tensor("wqk", [C, 512], BF16, kind="ExternalInput").ap()
    wv_d = nc.dram_tensor("wv", [C, 256], BF16, kind="ExternalInput").ap()
    wpT_d = nc.dram_tensor("wpT", [256, C], BF16, kind="ExternalInput").ap()
    bqk_d = nc.dram_tensor("bqk", [4, 128], F32, kind="ExternalInput").ap()
    bv_d = nc.dram_tensor("bv", [1, 256], F32, kind="ExternalInput").ap()
    cos_d = nc.dram_tensor("cosT", [128, T], BF16, kind="ExternalInput").ap()
    sin_d = nc.dram_tensor("sinT", [128, T], BF16, kind="ExternalInput").ap()
    psw_d = nc.dram_tensor("pswapT", [128, 128], BF16, kind="ExternalInput").ap()
    out_d = nc.dram_tensor("out", [T, C], BF16, kind="ExternalOutput").ap()

    with tile.TileContext(nc) as tc:
        # HAM warmup: ~20 back-to-back matmuls on a zeroed tile keep the PE
        # busy through the clock-gate's 4096-cycle activity window while the
        # startup DMAs stream, so real matmuls start at 2.4 GHz. Scoped so
        # the psum bank is released before the main pools allocate.
        with (
            tc.tile_pool(name="wuppool", bufs=1) as wup,
            tc.tile_pool(name="wpsum", bufs=1, space="PSUM") as wps,
        ):
            wz = wup.tile([128, 512], BF16)
            nc.gpsimd.memset(wz[:], 0.0)
            wps_t = wps.tile([128, 512], F32)
            for _ in range(20):
                nc.tensor.matmul(wps_t[:], wz[:, 0:128], wz[:], start=True, stop=True)

        with (
            tc.tile_pool(name="const", bufs=1) as const,
            tc.tile_pool(name="wqkp", bufs=1) as wqkp,
            tc.tile_pool(name="x0p", bufs=1) as x0p,
            tc.tile_pool(name="rotp", bufs=1) as rotp,
            tc.tile_pool(name="vsbp", bufs=1) as vsbp,
            tc.tile_pool(name="ptp", bufs=4) as ptp,
            tc.tile_pool(name="ypairp", bufs=1) as ypairp,
            tc.tile_pool(name="ysbp", bufs=2) as ysbp,
            tc.tile_pool(name="lrowp", bufs=2) as lrowp,
            tc.tile_pool(name="bcp", bufs=2) as bcp,
            tc.tile_pool(name="dumexp", bufs=1) as dumexp,
        ):
            psw_sb = const.tile([128, 128], BF16)
            cos_sb = const.tile([128, T], BF16)
            sin_sb = const.tile([128, T], BF16)
            bqk_sb = const.tile([128, 4], F32)
            bv_row = const.tile([1, 256], F32)
            bv_bc = const.tile([128, 256], F32)
            wpT_sb = const.tile([128, 2, C], BF16)

            # prime the ScalarE exp table set during the initial DMA wait
            dum = dumexp.tile([1, 8], F32)
            nc.vector.memset(dum[:], 0.0)
            nc.scalar.activation(
                out=dum[:], in_=dum[:], func=mybir.ActivationFunctionType.Exp
            )

            wqk_sb = wqkp.tile([128, 8, 512], BF16)
            x0_sb = x0p.tile([128, 8, SC], BF16)
            wqk_r = wqk_d.rearrange("(a b) c -> b a c", b=128)
            wv_r = wv_d.rearrange("(a b) c -> b a c", b=128)
            xT_r = xT_d.rearrange("(a b) c -> b a c", b=128)

            wv_sb = const.tile([128, 8, 256], BF16)

            # Startup critical path: the first qk m-pair needs wqk cols
            # 0:256 + all of x0. x0 is split across the scalar+vector
            # queues so two HW rings stream it; everything non-critical
            # (wv, cos/sin, xch1) is ordered after or gated.
            nc.sync.dma_start(out=wqk_sb[:, :, 0:256], in_=wqk_r[:, :, 0:256])
            nc.scalar.dma_start(out=x0_sb[:, 0:4, :], in_=xT_r[:, 0:4, 0:SC])
            nc.gpsimd.dma_start(out=x0_sb[:, 4:8, :], in_=xT_r[:, 4:8, 0:SC])
            nc.sync.dma_start(out=wqk_sb[:, :, 256:512], in_=wqk_r[:, :, 256:512])
            nc.gpsimd.dma_start(out=bqk_sb[:], in_=bqk_d.rearrange("a b -> b a"))
            nc.gpsimd.dma_start(out=bv_row[:], in_=bv_d[:, :])
            nc.gpsimd.dma_start(out=psw_sb[:], in_=psw_d[:, :])
            nc.scalar.dma_start(out=cos_sb[:, 0:1024], in_=cos_d[:, 0:1024])
            nc.scalar.dma_start(out=sin_sb[:, 0:1024], in_=sin_d[:, 0:1024])
            nc.sync.dma_start(out=wv_sb[:], in_=wv_r[:, :, :])
            nc.gpsimd.partition_broadcast(bv_bc[:, :], bv_row[0:1, :])

            # qT/kT after rope: m=0,1 q head-pairs; m=2,3 k head-pairs
            rot = [
                rotp.tile([128, T], BF16, tag=f"rot{m}", name=f"rot{m}")
                for m in range(4)
            ]
            # v with ones column per head: [128part(t), NSB, HPC*65]
            v_sb = vsbp.tile([128, NSB, HPC * 65], BF16)
            nc.vector.memset(v_sb[:], 1.0)
            ypair = [
                [
                    ypairp.tile(
                        [128, SC], BF16, tag=f"yp{tci}{p}", name=f"yp{tci}{p}"
                    )
                    for p in range(2)
                ]
                for tci in range(NT)
            ]

            def attn_chunk(tci, ps_pool, psy_pool, halves=((0, SC),)):
                """Scores+softmax+p@v+normalize for one 512-wide t-chunk.

                halves: column sub-ranges of the chunk; p@v accumulation and
                normalization are done per half so the earlier half's
                normalize/cproj can overlap the later half's matmuls.
                """
                t0 = tci * SC
                nsb = tci * 4 + 4
                for p in range(2):
                    psy = psy_pool.tile([65, 2, SC], F32, tag="psy", name="psy")
                    started = set()
                    for sbi in range(nsb):
                        s0 = sbi * 128
                        ssl = bass.ds(s0, 128)
                        # cols below d0 are causally dead: never computed
                        d0 = max(0, s0 - t0)
                        nn = SC - d0
                        pss = ps_pool.tile([128, 2 * SC], F32, tag="pss", name="pss")
                        nc.tensor.matmul(
                            pss[:, d0:SC],
                            rot[2 + p][0:64, ssl],
                            rot[p][0:64, bass.ds(t0 + d0, nn)],
                            tile_position=(0, 0),
                        )
                        nc.tensor.matmul(
                            pss[:, SC + d0 : 2 * SC],
                            rot[2 + p][64:128, ssl],
                            rot[p][64:128, bass.ds(t0 + d0, nn)],
                            tile_position=(64, 0),
                        )
                        pt = ptp.tile([128, 2 * SC], BF16, tag="pt", name="pt")
                        pt3 = pt[:].rearrange("p (h c) -> p h c", h=2)[:, :, d0:SC]
                        nc.scalar.activation(
                            out=pt3,
                            in_=pss[:].rearrange("p (h c) -> p h c", h=2)[:, :, d0:SC],
                            func=mybir.ActivationFunctionType.Exp,
                            scale=1.0 / math.sqrt(D),
                        )
                        if s0 >= t0:
                            # zero t < s for both heads: keep y' - x >= 0.
                            # only the first 128 cols past the diagonal can
                            # violate causality (x <= 127), so mask just those
                            pt3m = pt3[:, :, 0:128]
                            nc.gpsimd.affine_select(
                                out=pt3m,
                                in_=pt3m,
                                compare_op=mybir.AluOpType.is_ge,
                                fill=0.0,
                                base=0,
                                pattern=[[0, 2], [1, 128]],
                                channel_multiplier=-1,
                            )
                        for q in range(2):
                            h = 2 * p + q
                            for h0, h1 in halves:
                                c0 = max(h0, d0)
                                if c0 >= h1:
                                    continue
                                last_sbi = (t0 + h1) // 128 - 1
                                nc.tensor.matmul(
                                    psy[:, q, c0:h1],
                                    v_sb[:, sbi, h * 65 : h * 65 + 65],
                                    pt[:, q * SC + c0 : q * SC + h1],
                                    start=((q, h0) not in started),
                                    stop=(sbi == last_sbi),
                                )
                                started.add((q, h0))
                    for h0, h1 in halves:
                        w = h1 - h0
                        # free the psum bank right away; l-pipeline from SBUF
                        ysb = ysbp.tile([65, 2, w], F32, tag="ysb", name="ysb")
                        nc.vector.tensor_copy(ysb[:, :, :], psy[:, :, h0:h1])
                        # partition-shifted copy is HW-safe; a partition-
                        # shifted reciprocal is NOT (garbage on HW, ok in sim)
                        lraw = lrowp.tile([1, 2 * w], F32, tag="lraw", name="lraw")
                        lr3 = lraw[:].rearrange("p (a b) -> p a b", a=2)
                        nc.vector.tensor_copy(lr3[0:1, :, :], ysb[64:65, :, :])
                        lrow0 = lrowp.tile([1, 2 * w], F32, tag="lrow0", name="lrow0")
                        nc.vector.reciprocal_approx_fast(lrow0[0:1, :], lraw[0:1, :])
                        bc = bcp.tile([64, 2 * w], F32, tag="bc", name="bc")
                        nc.gpsimd.partition_broadcast(bc[:, :], lrow0[0:1, :])
                        for q in range(2):
                            # partition-shifted (q=1 writes parts 64:128 from
                            # parts 0:64 inputs): DVE only, not gpsimd
                            nc.vector.tensor_mul(
                                ypair[tci][p][q * 64 : (q + 1) * 64, h0:h1],
                                ysb[0:64, q, :],
                                bc[:, bass.ds(q * w, w)],
                            )

            def cproj_chunk(tci, pso_pool, ostp):
                t0 = tci * SC
                for ms in range(4):
                    ost = ostp.tile([128, C], BF16, tag="ost", name="ost")
                    for nch2 in range(2):
                        pso = pso_pool.tile([128, 512], F32, tag="pa", name="pso")
                        for kp in range(2):
                            nc.tensor.matmul(
                                pso[:],
                                ypair[tci][kp][:, bass.ts(ms, 128)],
                                wpT_sb[:, kp, bass.ts(nch2, 512)],
                                start=(kp == 0),
                                stop=(kp == 1),
                            )
                        if nch2 == 0:
                            nc.scalar.copy(ost[:, 0:512], pso[:])
                        else:
                            nc.vector.tensor_copy(ost[:, 512:1024], pso[:])
                        nc.sync.dma_start(
                            out=out_d[
                                bass.ds(t0 + ms * 128, 128), bass.ts(nch2, 512)
                            ],
                            in_=ost[:, bass.ts(nch2, 512)],
                        )

            # ---- Single schedule scope: projection, attention, and c_proj
            # share pools so the scheduler can weave them with no stage
            # barrier. PSUM: psA(2) + psE(2x2) + psyE(2) = 8 banks.
            with (
                tc.tile_pool(name="xchp", bufs=3) as xchp,
                tc.tile_pool(name="rawp", bufs=5) as rawp,
                tc.tile_pool(name="ttmp", bufs=3) as ttmp,
                tc.tile_pool(name="ostp", bufs=3) as ostp,
                tc.tile_pool(name="psA", bufs=2, space="PSUM") as psA,
                tc.tile_pool(name="psE", bufs=2, space="PSUM") as psE,
                tc.tile_pool(name="psyE", bufs=1, space="PSUM") as psyE,
            ):

                def proj_chunk(nch, rhs_of, vstat_of):
                    sl = bass.ts(nch, SC)
                    # q,k projection: out[m-tile, t-chunk]
                    raw = [
                        rawp.tile([128, SC], BF16, tag="raw", name=f"raw{m}")
                        for m in range(4)
                    ]
                    for m in range(4):
                        ps = psA.tile([128, SC], F32, tag="pa", name="pa")
                        for ct in range(8):
                            nc.tensor.matmul(
                                ps[:],
                                wqk_sb[:, ct, bass.ts(m, 128)],
                                rhs_of(ct),
                                start=(ct == 0),
                                stop=(ct == 7),
                            )
                        nc.vector.tensor_scalar_add(
                            raw[m], ps[:], bqk_sb[:, m : m + 1]
                        )
                    # v projection for the 4 t-subtiles of this chunk
                    for tml in range(4):
                        tm = nch * 4 + tml
                        psv = psA.tile([128, 256], F32, tag="pa", name="pav")
                        for ct in range(8):
                            nc.tensor.matmul(
                                psv[:],
                                vstat_of(ct, tml),
                                wv_sb[:, ct, :],
                                start=(ct == 0),
                                stop=(ct == 7),
                            )
                        nc.vector.tensor_add(
                            v_sb[:, tm, :]
                            .rearrange("p (h c) -> p h c", h=HPC)[:, :, 0:64],
                            psv[:].rearrange("p (h c) -> p h c", h=HPC),
                            bv_bc[:].rearrange("p (h c) -> p h c", h=HPC),
                        )
                    # rope on the 4 qk tiles for this chunk
                    for m in range(4):
                        psw = psA.tile([128, SC], F32, tag="pa", name="paw")
                        nc.tensor.matmul(psw[:], psw_sb[:], raw[m][:])
                        tmp = ttmp.tile([128, SC], BF16, tag="ttmp")
                        nc.vector.tensor_mul(tmp[:], psw[:], sin_sb[:, sl])
                        nc.vector.tensor_mul(rot[m][:, sl], raw[m][:], cos_sb[:, sl])
                        nc.vector.tensor_add(rot[m][:, sl], rot[m][:, sl], tmp[:])

                # xch1/2/3 and wpT are issued from the gpsimd engine stream
                # (gated) so their transfers don't steal HBM bandwidth from
                # the startup-critical loads.
                xchs = {1: xchp.tile([128, 8, SC], BF16, tag="xch", name="xch1")}
                with tc.tile_wait_until(0.008):
                    nc.gpsimd.dma_start(
                        out=xchs[1][:], in_=xT_r[:, :, bass.ts(1, SC)]
                    )
                for nch in range(4):
                    if nch == 0:
                        rhs_of = lambda ct: x0_sb[:, ct, :]
                        vstat_of = lambda ct, tml: x0_sb[:, ct, bass.ts(tml, 128)]
                    else:
                        xch = xchs[nch]
                        rhs_of = lambda ct, xch=xch: xch[:, ct, :]
                        vstat_of = lambda ct, tml, xch=xch: xch[
                            :, ct, bass.ts(tml, 128)
                        ]
                    proj_chunk(nch, rhs_of, vstat_of)
                    # weave in attention as soon as its proj chunk is done
                    # (chunk tci needs proj 0..tci); exp fills ScalarE while
                    # the PE is projection-bound. c_proj of the previous
                    # chunk gives the PE work while exp runs.
                    if nch < 3:
                        attn_chunk(nch, psE, psyE)
                    if nch == 0:
                        # gated loads: pinned after attn0's gpsimd work both
                        # in static order (tile_wait_until) and at runtime
                        # (in-order gpsimd stream)
                        with tc.tile_wait_until(0.016):
                            nc.gpsimd.dma_start(
                                out=cos_sb[:, 1024:2048], in_=cos_d[:, 1024:2048]
                            )
                            nc.gpsimd.dma_start(
                                out=sin_sb[:, 1024:2048], in_=sin_d[:, 1024:2048]
                            )
                            nc.gpsimd.dma_start(
                                out=wpT_sb[:],
                                in_=wpT_d.rearrange("(a b) c -> b a c", b=128),
                            )
                    if nch + 2 <= 3:
                        xchs[nch + 2] = xchp.tile(
                            [128, 8, SC], BF16, tag="xch", name=f"xch{nch + 2}"
                        )
                        with tc.tile_wait_until(0.018 + 0.008 * nch):
                            nc.gpsimd.dma_start(
                                out=xchs[nch + 2][:],
                                in_=xT_r[:, :, bass.ts(nch + 2, SC)],
                            )
                    if nch >= 1:
                        cproj_chunk(nch - 1, psA, ostp)
                attn_chunk(3, psE, psyE)
                cproj_chunk(3, psA, ostp)

    nc.compile()
    return nc


def _host_shards(x, w_attn, b_attn, w_proj):
    """Per-core input dicts. Core c: batch c//TP, heads [HPC*(c%TP) .. )."""
    pos = np.arange(T, dtype=np.float64)
    div = np.exp(np.arange(0, D, 2, dtype=np.float64) * (-(math.log(10000.0) / D)))
    sinu = np.outer(pos, div)  # [T, 32]
    bf = ml_dtypes.bfloat16
    cosT = np.tile(np.cos(sinu).T, (4, 1)).astype(bf)  # [128, T]
    sinT = np.tile(np.sin(sinu).T, (4, 1)).astype(bf)

    psw = np.zeros((128, 128), dtype=np.float32)  # P[out,in]
    for blk in (0, 64):
        for j in range(32):
            psw[blk + j, blk + 32 + j] = -1.0
            psw[blk + 32 + j, blk + j] = 1.0
    pswapT = np.ascontiguousarray(psw.T).astype(bf)

    ev = np.arange(0, D, 2)
    od = np.arange(1, D, 2)
    in_maps = []
    for c in range(N_CORES):
        b, lane = c // TP, c % TP
        heads = [HPC * lane + i for i in range(HPC)]
        idx_qk = []
        for off in (0, C):  # q rows then k rows, deinterleaved per head
            for p in range(2):
                for hh in (heads[2 * p], heads[2 * p + 1]):
                    base = off + hh * D
                    idx_qk.extend((base + ev).tolist())
                    idx_qk.extend((base + od).tolist())
        idx_qk = np.array(idx_qk)
        idx_v = np.concatenate([2 * C + h * D + np.arange(D) for h in heads])
        cols_p = np.concatenate([h * D + np.arange(D) for h in heads])
        in_maps.append(
            {
                "xT": np.ascontiguousarray(x[b].T).astype(bf),
                "wqk": np.ascontiguousarray(w_attn[idx_qk, :].T).astype(bf),
                "wv": np.ascontiguousarray(w_attn[idx_v, :].T).astype(bf),
                "wpT": np.ascontiguousarray(w_proj[:, cols_p].T).astype(bf),
                "bqk": np.ascontiguousarray(b_attn[idx_qk].reshape(4, 128)),
                "bv": np.ascontiguousarray(b_attn[idx_v].reshape(1, 256)),
                "cosT": cosT,
                "sinT": sinT,
                "pswapT": pswapT,
            }
        )
    return in_maps


def kernel(x, w_attn, b_attn, w_proj, b_proj, _trace=False):
    x = np.asarray(x, dtype=np.float32)
    w_attn = np.asarray(w_attn, dtype=np.float32)
    b_attn = np.asarray(b_attn, dtype=np.float32)
    w_proj = np.asarray(w_proj, dtype=np.float32)
    b_proj = np.asarray(b_proj, dtype=np.float32)

    if "nc" not in _cached:
        _cached["nc"] = _build_program()
    nc = _cached["nc"]

    in_maps = _host_shards(x, w_attn, b_attn, w_proj)
    res = bass_utils.run_bass_kernel_spmd(
        nc, in_maps, core_ids=list(range(N_CORES)), trace=_trace
    )
    _cached["last_result"] = res

    out = np.empty((B, T, C), dtype=np.float32)
    for b in range(B):
        acc = res.results[b * TP]["out"].astype(np.float32)
        for lane in range(1, TP):
            acc = acc + res.results[b * TP + lane]["out"].astype(np.float32)
        out[b] = acc + b_proj[None, :]
    return out


# revision 10
# speedup vs baseline: 1.0065x; 1.0065x over previous
"""Causal self-attention (B=2, T=2048, C=1024, H=16) on 8 Trainium2 cores.

Sharding: DP2 over batch x TP4 over heads (4 heads/core). Each core computes
its batch's QKV projection for its heads, RoPE, causal attention, and a
partial c_proj over its 256 input channels. Host sums the 4 partials per
batch and adds b_proj.

v5 (from v2 @ 181.7us):
- Startup: x0 is DMA'd as 8 per-ct slices (scalar queue ct0-3, gpsimd ct4-7)
  and chunk 0's qk projection runs ct-outer over m-pairs, so the first
  matmul starts as soon as wqk[m01]+x0[ct0] land (~9us) instead of waiting
  for the full x0 (~15us). A short warmup matmul burst bridges the HAM
  clock-gate window so compute runs at 2.4 GHz.
- RoPE batched per m-pair: raw/tmp/rot live in [128,2,*] pair tiles; the
  cos-mul and sin-add run as single [128,2,512] DVE ops (cos/sin broadcast
  via stride-0). The psum->sbuf bias conversions split scalar/vector.
- Softmax normalize batched per (chunk, head-pair): one ysb copy, one
  reciprocal, one partition_broadcast.
- Chunk 3 (the tail) runs p@v in two column phases (0:256 over s-blocks
  0..13, then 256:512 over all 16): phase A's normalize and cproj(ms 0,1)
  overlap phase B's matmuls. pt tiles for chunk 3 are kept live (bufs=16)
  so phase B re-reads exp'd scores.
- Output tiles DMA per 512-half as soon as each half's copy lands.

Scores are computed two heads at a time via tile_position row packing, exp
runs on ScalarE straight from PSUM with the 1/sqrt(D) scale fused, and the
causal mask is an affine_select on diagonal blocks only. V carries a ones
column per head so the softmax denominator falls out of the p@v matmul.
"""

import sys

sys.path.insert(0, "/opt/trn_rl_repo")

import math

import ml_dtypes
import numpy as np

import concourse.bass as bass
import concourse.mybir as mybir
import concourse.tile as tile
from concourse import bacc, bass_utils

B, T, C = 2, 2048, 1024
H, D = 16, 64
N_CORES = 8
DP, TP = 2, 4
HPC = H // TP  # heads per core
SC = 512  # t-chunk width / psum bank width
NT = T // SC
NSB = T // 128  # s-blocks

F32 = mybir.dt.float32
BF16 = mybir.dt.bfloat16

_cached = {}


def _build_program():
    nc = bacc.Bacc("TRN2", target_bir_lowering=False, debug=False, num_devices=N_CORES)

    xT_d = nc.dram_tensor("xT", [C, T], BF16, kind="ExternalInput").ap()
    wqk_d = nc.dram_tensor("wqk", [C, 512], BF16, kind="ExternalInput").ap()
    wv_d = nc.dram_tensor("wv", [C, 256], BF16, kind="ExternalInput").ap()
    wpT_d = nc.dram_tensor("wpT", [256, C], BF16, kind="ExternalInput").ap()
    bqk_d = nc.dram_tensor("bqk", [4, 128], F32, kind="ExternalInput").ap()
    bv_d = nc.dram_tensor("bv", [1, 256], F32, kind="ExternalInput").ap()
    cos_d = nc.dram_tensor("cosT", [128, T], BF16, kind="ExternalInput").ap()
    sin_d = nc.dram_tensor("sinT", [128, T], BF16, kind="ExternalInput").ap()
    psw_d = nc.dram_tensor("pswapT", [128, 128], BF16, kind="ExternalInput").ap()
    out_d = nc.dram_tensor("out", [T, C], BF16, kind="ExternalOutput").ap()

    with tile.TileContext(nc) as tc:
        # HAM warmup: a few back-to-back matmuls bridge the clock-gate's
        # activity window between program start and the first DMA-fed
        # matmul, so real compute runs at 2.4 GHz from the start.
        with (
            tc.tile_pool(name="wuppool", bufs=1) as wup,
            tc.tile_pool(name="wpsum", bufs=1, space="PSUM") as wps,
        ):
            wz = wup.tile([128, 256], BF16)
            nc.gpsimd.memset(wz[:], 0.0)
            wps_t = wps.tile([128, 256], F32)
            for _ in range(8):
                nc.tensor.matmul(wps_t[:], wz[:, 0:128], wz[:], start=True, stop=True)

        with (
            tc.tile_pool(name="const", bufs=1) as const,
            tc.tile_pool(name="wqkp", bufs=1) as wqkp,
            tc.tile_pool(name="x0p", bufs=1) as x0p,
            tc.tile_pool(name="rotp", bufs=1) as rotp,
            tc.tile_pool(name="vsbp", bufs=1) as vsbp,
            tc.tile_pool(name="ptp", bufs=4) as ptp,
            tc.tile_pool(name="ypairp", bufs=1) as ypairp,
            tc.tile_pool(name="ysbp", bufs=2) as ysbp,
            tc.tile_pool(name="lrowp", bufs=2) as lrowp,
            tc.tile_pool(name="bcp", bufs=2) as bcp,
            tc.tile_pool(name="dumexp", bufs=1) as dumexp,
        ):
            psw_sb = const.tile([128, 128], BF16)
            cos_sb = const.tile([128, T], BF16)
            sin_sb = const.tile([128, T], BF16)
            bqk_sb = const.tile([128, 4], F32)
            bv_row = const.tile([1, 256], F32)
            bv_bc = const.tile([128, 256], F32)
            wpT_sb = const.tile([128, 2, C], BF16)

            # prime the ScalarE exp table set during the initial DMA wait
            dum = dumexp.tile([1, 8], F32)
            nc.vector.memset(dum[:], 0.0)
            nc.scalar.activation(
                out=dum[:], in_=dum[:], func=mybir.ActivationFunctionType.Exp
            )

            wqk_sb = wqkp.tile([128, 8, 512], BF16)
            x0_sb = x0p.tile([128, 8, SC], BF16)
            wqk_r = wqk_d.rearrange("(a b) c -> b a c", b=128)
            wv_r = wv_d.rearrange("(a b) c -> b a c", b=128)
            xT_r = xT_d.rearrange("(a b) c -> b a c", b=128)

            wv_sb = const.tile([128, 8, 256], BF16)

            # Startup critical path: chunk 0 runs ct-outer, so the first
            # matmul needs only wqk[m01, ct0-3] + x0[ct0]. x0 arrives as 8
            # per-ct slices on two queues; everything non-critical is
            # ordered after or gated onto the gpsimd stream.
            nc.sync.dma_start(out=wqk_sb[:, 0:4, 0:256], in_=wqk_r[:, 0:4, 0:256])
            nc.scalar.dma_start(out=x0_sb[:, 0, :], in_=xT_r[:, 0, 0:SC])
            nc.gpsimd.dma_start(out=x0_sb[:, 4, :], in_=xT_r[:, 4, 0:SC])
            nc.sync.dma_start(out=wqk_sb[:, 4:8, 0:256], in_=wqk_r[:, 4:8, 0:256])
            for ct in range(1, 4):
                nc.scalar.dma_start(out=x0_sb[:, ct, :], in_=xT_r[:, ct, 0:SC])
                nc.gpsimd.dma_start(
                    out=x0_sb[:, ct + 4, :], in_=xT_r[:, ct + 4, 0:SC]
                )
            nc.sync.dma_start(out=wqk_sb[:, :, 256:512], in_=wqk_r[:, :, 256:512])
            nc.gpsimd.dma_start(out=bqk_sb[:], in_=bqk_d.rearrange("a b -> b a"))
            nc.gpsimd.dma_start(out=bv_row[:], in_=bv_d[:, :])
            nc.gpsimd.dma_start(out=psw_sb[:], in_=psw_d[:, :])
            nc.scalar.dma_start(out=cos_sb[:, 0:1024], in_=cos_d[:, 0:1024])
            nc.scalar.dma_start(out=sin_sb[:, 0:1024], in_=sin_d[:, 0:1024])
            nc.sync.dma_start(out=wv_sb[:], in_=wv_r[:, :, :])
            nc.gpsimd.partition_broadcast(bv_bc[:, :], bv_row[0:1, :])

            # qT/kT after rope, as m-pair tiles: rot2[0] = q pairs (m0,m1),
            # rot2[1] = k pairs (m2,m3). rot2[i][:, j, :] is one m-tile.
            rot2 = [
                rotp.tile([128, 2, T], BF16, tag=f"rot{i}", name=f"rot{i}")
                for i in range(2)
            ]
            # v with ones column per head: [128part(t), NSB, HPC*65]
            v_sb = vsbp.tile([128, NSB, HPC * 65], BF16)
            nc.gpsimd.memset(v_sb[:], 1.0)
            ypair = [
                [
                    ypairp.tile(
                        [128, SC], BF16, tag=f"yp{tci}{p}", name=f"yp{tci}{p}"
                    )
                    for p in range(2)
                ]
                for tci in range(NT)
            ]

            def score_exp(tci, p, sbi, ps_pool, pt_pool, pt_tag, pt_bufs):
                """Scores + exp (+causal mask) for one 128-wide s-block."""
                t0 = tci * SC
                s0 = sbi * 128
                ssl = bass.ds(s0, 128)
                d0 = max(0, s0 - t0)
                nn = SC - d0
                pss = ps_pool.tile([128, 2 * SC], F32, tag="pss", name="pss")
                nc.tensor.matmul(
                    pss[:, d0:SC],
                    rot2[1][0:64, p, ssl],
                    rot2[0][0:64, p, bass.ds(t0 + d0, nn)],
                    tile_position=(0, 0),
                )
                nc.tensor.matmul(
                    pss[:, SC + d0 : 2 * SC],
                    rot2[1][64:128, p, ssl],
                    rot2[0][64:128, p, bass.ds(t0 + d0, nn)],
                    tile_position=(64, 0),
                )
                pt = pt_pool.tile(
                    [128, 2 * SC], BF16, tag=pt_tag, bufs=pt_bufs, name="pt"
                )
                pt3 = pt[:].rearrange("p (h c) -> p h c", h=2)[:, :, d0:SC]
                nc.scalar.activation(
                    out=pt3,
                    in_=pss[:].rearrange("p (h c) -> p h c", h=2)[:, :, d0:SC],
                    func=mybir.ActivationFunctionType.Exp,
                    scale=1.0 / math.sqrt(D),
                )
                if s0 >= t0:
                    # zero t < s for both heads: only the first 128 cols past
                    # the diagonal can violate causality, so mask just those
                    pt3m = pt3[:, :, 0:128]
                    nc.gpsimd.affine_select(
                        out=pt3m,
                        in_=pt3m,
                        compare_op=mybir.AluOpType.is_ge,
                        fill=0.0,
                        base=0,
                        pattern=[[0, 2], [1, 128]],
                        channel_multiplier=-1,
                    )
                return pt

            def pv_block(tci, p, sbi, pt, psy, h0, h1, first, last_sbi):
                """p@v contribution of s-block sbi to columns [h0,h1)."""
                t0 = tci * SC
                d0 = max(0, sbi * 128 - t0)
                c0 = max(h0, d0)
                if c0 >= h1:
                    return False
                for q in range(2):
                    h = 2 * p + q
                    nc.tensor.matmul(
                        psy[:, q, c0:h1],
                        v_sb[:, sbi, h * 65 : h * 65 + 65],
                        pt[:, q * SC + c0 : q * SC + h1],
                        start=first,
                        stop=(sbi == last_sbi),
                    )
                return True

            def normalize(tci, p, psy, h0, h1):
                """psum y' -> ypair with softmax denominator, cols [h0,h1)."""
                w = h1 - h0
                # free the psum bank right away; l-pipeline from SBUF
                ysb = ysbp.tile([65, 2, w], F32, tag="ysb", name="ysb")
                nc.vector.tensor_copy(ysb[:, :, :], psy[:, :, h0:h1])
                # partition-shifted copy is HW-safe; a partition-shifted
                # reciprocal is NOT (garbage on HW, fine in sim)
                lraw = lrowp.tile([1, 2 * w], F32, tag="lraw", name="lraw")
                lr3 = lraw[:].rearrange("p (a b) -> p a b", a=2)
                nc.vector.tensor_copy(lr3[0:1, :, :], ysb[64:65, :, :])
                lrow0 = lrowp.tile([1, 2 * w], F32, tag="lrow0", name="lrow0")
                nc.vector.reciprocal_approx_fast(lrow0[0:1, :], lraw[0:1, :])
                bc = bcp.tile([64, 2 * w], F32, tag="bc", name="bc")
                nc.gpsimd.partition_broadcast(bc[:, :], lrow0[0:1, :])
                for q in range(2):
                    nc.vector.tensor_mul(
                        ypair[tci][p][q * 64 : (q + 1) * 64, h0:h1],
                        ysb[0:64, q, :],
                        bc[:, bass.ds(q * w, w)],
                    )

            def attn_chunk(tci, ps_pool, psy_pool):
                nsb = tci * 4 + 4
                for p in range(2):
                    psy = psy_pool.tile([65, 2, SC], F32, tag="psy", name="psy")
                    for sbi in range(nsb):
                        pt = score_exp(tci, p, sbi, ps_pool, ptp, "pt", None)
                        pv_block(tci, p, sbi, pt, psy, 0, SC, sbi == 0, nsb - 1)
                    normalize(tci, p, psy, 0, SC)

            def cproj_chunk(tci, pso_pool, ostp, ms_list=(0, 1, 2, 3)):
                t0 = tci * SC
                for ms in ms_list:
                    ost = ostp.tile([128, C], BF16, tag="ost", name="ost")
                    for nch2 in range(2):
                        pso = pso_pool.tile([128, 512], F32, tag="pa", name="pso")
                        for kp in range(2):
                            nc.tensor.matmul(
                                pso[:],
                                ypair[tci][kp][:, bass.ts(ms, 128)],
                                wpT_sb[:, kp, bass.ts(nch2, 512)],
                                start=(kp == 0),
                                stop=(kp == 1),
                            )
                        if nch2 == 0:
                            nc.scalar.copy(ost[:, 0:512], pso[:])
                        else:
                            nc.vector.tensor_copy(ost[:, 512:1024], pso[:])
                        nc.sync.dma_start(
                            out=out_d[
                                bass.ds(t0 + ms * 128, 128), bass.ts(nch2, 512)
                            ],
                            in_=ost[:, bass.ts(nch2, 512)],
                        )

            # ---- Single schedule scope: projection, attention, and c_proj
            # share pools so the scheduler can weave them with no stage
            # barrier. PSUM: psA(2) + psE(2x2) + psyE(2) = 8 banks.
            with (
                tc.tile_pool(name="xchp", bufs=3) as xchp,
                tc.tile_pool(name="rawp", bufs=3) as rawp,
                tc.tile_pool(name="ttmp", bufs=2) as ttmp,
                tc.tile_pool(name="ostp", bufs=3) as ostp,
                tc.tile_pool(name="psA", bufs=2, space="PSUM") as psA,
                tc.tile_pool(name="psE", bufs=2, space="PSUM") as psE,
                tc.tile_pool(name="psyE", bufs=1, space="PSUM") as psyE,
            ):

                def qk_mpair(mp, rhs_of, ct_outer):
                    """qk projection m-pair -> raw2 pair tile (bias added)."""
                    raw2 = rawp.tile([128, 2, SC], BF16, tag="raw", name="raw2")
                    if ct_outer:
                        ps = [
                            psA.tile([128, SC], F32, tag="pa", name=f"pa{j}")
                            for j in range(2)
                        ]
                        for ct in range(8):
                            for j in range(2):
                                nc.tensor.matmul(
                                    ps[j][:],
                                    wqk_sb[:, ct, bass.ts(2 * mp + j, 128)],
                                    rhs_of(ct),
                                    start=(ct == 0),
                                    stop=(ct == 7),
                                )
                    else:
                        ps = []
                        for j in range(2):
                            psj = psA.tile([128, SC], F32, tag="pa", name=f"pa{j}")
                            for ct in range(8):
                                nc.tensor.matmul(
                                    psj[:],
                                    wqk_sb[:, ct, bass.ts(2 * mp + j, 128)],
                                    rhs_of(ct),
                                    start=(ct == 0),
                                    stop=(ct == 7),
                                )
                            ps.append(psj)
                    # psum->sbuf + bias: one half on scalar, one on vector
                    m = 2 * mp
                    nc.scalar.activation(
                        out=raw2[:, 0, :],
                        in_=ps[0][:],
                        func=mybir.ActivationFunctionType.Identity,
                        bias=bqk_sb[:, m : m + 1],
                    )
                    nc.vector.tensor_scalar_add(
                        raw2[:, 1, :], ps[1][:], bqk_sb[:, m + 1 : m + 2]
                    )
                    return raw2

                def rope_mpair(mp, raw2, sl):
                    """RoPE for one m-pair: rot2 = raw2*cos + (P@raw2)*sin."""
                    tmp2 = ttmp.tile([128, 2, SC], BF16, tag="ttmp", name="tmp2")
                    for j in range(2):
                        psw = psA.tile([128, SC], F32, tag="pa", name="paw")
                        nc.tensor.matmul(psw[:], psw_sb[:], raw2[:, j, :])
                        nc.vector.tensor_mul(tmp2[:, j, :], psw[:], sin_sb[:, sl])
                    cos_b = (
                        cos_sb[:, sl]
                        .rearrange("p (a c) -> p a c", a=1)
                        .broadcast_to([128, 2, SC])
                    )
                    dst = rot2[mp][:, :, sl]
                    nc.vector.tensor_mul(dst, raw2[:], cos_b)
                    nc.vector.tensor_add(dst, dst, tmp2[:])

                def proj_chunk(nch, rhs_of, vstat_of, ct_outer=False):
                    sl = bass.ts(nch, SC)
                    raws = [qk_mpair(mp, rhs_of, ct_outer) for mp in range(2)]
                    # v projection for the 4 t-subtiles of this chunk
                    for tml in range(4):
                        tm = nch * 4 + tml
                        psv = psA.tile([128, 256], F32, tag="pa", name="pav")
                        for ct in range(8):
                            nc.tensor.matmul(
                                psv[:],
                                vstat_of(ct, tml),
                                wv_sb[:, ct, :],
                                start=(ct == 0),
                                stop=(ct == 7),
                            )
                        nc.vector.tensor_add(
                            v_sb[:, tm, :]
                            .rearrange("p (h c) -> p h c", h=HPC)[:, :, 0:64],
                            psv[:].rearrange("p (h c) -> p h c", h=HPC),
                            bv_bc[:].rearrange("p (h c) -> p h c", h=HPC),
                        )
                    for mp in range(2):
                        rope_mpair(mp, raws[mp], sl)

                # xch1/2/3 and wpT are issued from the gpsimd engine stream
                # (gated) so their transfers don't steal HBM bandwidth from
                # the startup-critical loads.
                xchs = {1: xchp.tile([128, 8, SC], BF16, tag="xch", name="xch1")}
                with tc.tile_wait_until(0.007):
                    nc.gpsimd.dma_start(
                        out=xchs[1][:], in_=xT_r[:, :, bass.ts(1, SC)]
                    )
                for nch in range(4):
                    if nch == 0:
                        rhs_of = lambda ct: x0_sb[:, ct, :]
                        vstat_of = lambda ct, tml: x0_sb[:, ct, bass.ts(tml, 128)]
                    else:
                        xch = xchs[nch]
                        rhs_of = lambda ct, xch=xch: xch[:, ct, :]
                        vstat_of = lambda ct, tml, xch=xch: xch[
                            :, ct, bass.ts(tml, 128)
                        ]
                    proj_chunk(nch, rhs_of, vstat_of, ct_outer=(nch == 0))
                    # weave in attention as soon as its proj chunk is done
                    # (chunk tci needs proj 0..tci); exp fills ScalarE while
                    # the PE is projection-bound. c_proj of the previous
                    # chunk gives the PE work while exp runs.
                    if nch < 3:
                        attn_chunk(nch, psE, psyE)
                    if nch == 0:
                        with tc.tile_wait_until(0.015):
                            nc.gpsimd.dma_start(
                                out=cos_sb[:, 1024:2048], in_=cos_d[:, 1024:2048]
                            )
                            nc.gpsimd.dma_start(
                                out=sin_sb[:, 1024:2048], in_=sin_d[:, 1024:2048]
                            )
                            nc.gpsimd.dma_start(
                                out=wpT_sb[:],
                                in_=wpT_d.rearrange("(a b) c -> b a c", b=128),
                            )
                    if nch + 2 <= 3:
                        xchs[nch + 2] = xchp.tile(
                            [128, 8, SC], BF16, tag="xch", name=f"xch{nch + 2}"
                        )
                        with tc.tile_wait_until(0.017 + 0.008 * nch):
                            nc.gpsimd.dma_start(
                                out=xchs[nch + 2][:],
                                in_=xT_r[:, :, bass.ts(nch + 2, SC)],
                            )
                    if nch >= 1:
                        cproj_chunk(nch - 1, psA, ostp)

                # ---- chunk 3 (tail): two-phase p@v so normalize + cproj of
                # cols 0:256 overlap the phase-B matmuls. pt tiles stay live
                # (bufs=16) so phase B re-reads them.
                for p in range(2):
                    psy = psyE.tile([65, 2, SC], F32, tag="psy", name="psy")
                    pts = []
                    for sbi in range(NSB):
                        pt = score_exp(3, p, sbi, psE, ptp, "pt3", 16)
                        pts.append(pt)
                        if sbi <= 13:
                            pv_block(3, p, sbi, pt, psy, 0, 256, sbi == 0, 13)
                    normalize(3, p, psy, 0, 256)
                    # phase B: new accumulation groups in the same banks; the
                    # bank-wide has_written clear at start only affects
                    # accumulation state, not phase A's data (already copied)
                    for sbi in range(NSB):
                        pv_block(3, p, sbi, pts[sbi], psy, 256, SC, sbi == 0, 15)
                    if p == 1:
                        cproj_chunk(3, psA, ostp, ms_list=(0, 1))
                    normalize(3, p, psy, 256, SC)
                cproj_chunk(3, psA, ostp, ms_list=(2, 3))

    nc.compile()
    return nc


def _host_shards(x, w_attn, b_attn, w_proj):
    """Per-core input dicts. Core c: batch c//TP, heads [HPC*(c%TP) .. )."""
    pos = np.arange(T, dtype=np.float64)
    div = np.exp(np.arange(0, D, 2, dtype=np.float64) * (-(math.log(10000.0) / D)))
    sinu = np.outer(pos, div)  # [T, 32]
    bf = ml_dtypes.bfloat16
    cosT = np.tile(np.cos(sinu).T, (4, 1)).astype(bf)  # [128, T]
    sinT = np.tile(np.sin(sinu).T, (4, 1)).astype(bf)

    psw = np.zeros((128, 128), dtype=np.float32)  # P[out,in]
    for blk in (0, 64):
        for j in range(32):
            psw[blk + j, blk + 32 + j] = -1.0
            psw[blk + 32 + j, blk + j] = 1.0
    pswapT = np.ascontiguousarray(psw.T).astype(bf)

    ev = np.arange(0, D, 2)
    od = np.arange(1, D, 2)
    in_maps = []
    for c in range(N_CORES):
        b, lane = c // TP, c % TP
        heads = [HPC * lane + i for i in range(HPC)]
        idx_qk = []
        for off in (0, C):  # q rows then k rows, deinterleaved per head
            for p in range(2):
                for hh in (heads[2 * p], heads[2 * p + 1]):
                    base = off + hh * D
                    idx_qk.extend((base + ev).tolist())
                    idx_qk.extend((base + od).tolist())
        idx_qk = np.array(idx_qk)
        idx_v = np.concatenate([2 * C + h * D + np.arange(D) for h in heads])
        cols_p = np.concatenate([h * D + np.arange(D) for h in heads])
        in_maps.append(
            {
                "xT": np.ascontiguousarray(x[b].T).astype(bf),
                "wqk": np.ascontiguousarray(w_attn[idx_qk, :].T).astype(bf),
                "wv": np.ascontiguousarray(w_attn[idx_v, :].T).astype(bf),
                "wpT": np.ascontiguousarray(w_proj[:, cols_p].T).astype(bf),
                "bqk": np.ascontiguousarray(b_attn[idx_qk].reshape(4, 128)),
                "bv": np.ascontiguousarray(b_attn[idx_v].reshape(1, 256)),
                "cosT": cosT,
                "sinT": sinT,
                "pswapT": pswapT,
            }
        )
    return in_maps


def kernel(x, w_attn, b_attn, w_proj, b_proj, _trace=False):
    x = np.asarray(x, dtype=np.float32)
    w_attn = np.asarray(w_attn, dtype=np.float32)
    b_attn = np.asarray(b_attn, dtype=np.float32)
    w_proj = np.asarray(w_proj, dtype=np.float32)
    b_proj = np.asarray(b_proj, dtype=np.float32)

    if "nc" not in _cached:
        _cached["nc"] = _build_program()
    nc = _cached["nc"]

    in_maps = _host_shards(x, w_attn, b_attn, w_proj)
    res = bass_utils.run_bass_kernel_spmd(
        nc, in_maps, core_ids=list(range(N_CORES)), trace=_trace
    )
    _cached["last_result"] = res

    out = np.empty((B, T, C), dtype=np.float32)
    for b in range(B):
        acc = res.results[b * TP]["out"].astype(np.float32)
        for lane in range(1, TP):
            acc = acc + res.results[b * TP + lane]["out"].astype(np.float32)
        out[b] = acc + b_proj[None, :]
    return out
